# revision 8
# baseline (speedup 1.0000x reference)
"""GCN layer (GCNConv + skip + BN + ReLU) on 8 TRN2 cores — v3.

vs baseline: self-loops removed from the gather (added via one identity
matmul per tile); edges packed per (tile-group, bank) with no per-tile
padding (boundary gather-columns get two masked S-columns) -> 200.6k gather
descriptors/core vs 250.9k; S one-hots built on DVE (fp32-in bf16-out,
batched x4) instead of 2 ACT ops per chunk; transform matmuls in bf16.
"""

import numpy as np
import ml_dtypes

P = 128
BANK_MAX = 32768

_BF16 = ml_dtypes.bfloat16

_KCACHE = {}


def _host_prep(x, edge_index, W, skip_W, gamma, beta, M, IN, OUT, GT):
    N = x.shape[0]
    SH = N // M
    T = -(-SH // P)
    SHP = T * P
    NFP = M * SHP
    NB = -(-NFP // BANK_MAX)
    BK = NFP // NB
    NG = -(-T // GT)
    assert NFP % NB == 0 and BK <= BANK_MAX

    row = edge_index[0].astype(np.int64)
    col = edge_index[1].astype(np.int64)
    loops = np.arange(N, dtype=np.int64)
    deg = np.bincount(np.concatenate([col, loops]),
                      minlength=N).astype(np.float32)

    # snake-balanced node -> (tile, slot)
    node_pos = np.empty(N, dtype=np.int64)
    for m in range(M):
        dg = deg[m * SH:(m + 1) * SH]
        order_n = np.argsort(-dg, kind="stable")
        ranks = np.empty(SH, dtype=np.int64)
        ranks[order_n] = np.arange(SH)
        rounds = ranks // T
        tpos = ranks % T
        tile_of = np.where(rounds % 2 == 0, tpos, T - 1 - tpos)
        node_pos[m * SH:(m + 1) * SH] = tile_of * P + rounds

    src_pad = (row // SH) * SHP + node_pos[row]
    bank = src_pad // BK
    core = col // SH
    post = node_pos[col]
    tile = post // P
    grp = (core * NG + tile // GT) * NB + bank
    key = grp * T + tile
    order = np.argsort(key, kind="stable")
    srcrow_s = (src_pad - bank * BK)[order]
    colloc_s = (post % P)[order]
    tile_s = tile[order]
    grp_s = grp[order]

    NGRP = M * NG * NB
    cnts = np.bincount(grp_s, minlength=NGRP)
    starts = np.zeros(NGRP + 1, dtype=np.int64)
    np.cumsum(cnts, out=starts[1:])

    # ---- UNION structure: identical program for every core ----
    # per (g, b): ncols_u = max over cores; per column: union of owner tiles
    uni_calls = []            # (g, b, nidx16, ncols)
    owners = []               # per call: list per ci of sorted owner tiles
    per_core_seg = {}         # (m, g, b) -> (rows_e, cols_e, tils_e)
    for g in range(NG):
        for b in range(NB):
            ncols_u = 0
            own = None
            for m in range(M):
                gi = (m * NG + g) * NB + b
                c = int(cnts[gi])
                s0 = starts[gi]
                per_core_seg[(m, g, b)] = (
                    srcrow_s[s0:s0 + c], colloc_s[s0:s0 + c],
                    tile_s[s0:s0 + c])
                ncols_u = max(ncols_u, -(-c // P))
            if ncols_u == 0:
                continue
            own = [set() for _ in range(ncols_u)]
            for m in range(M):
                (_, _, tils_e) = per_core_seg[(m, g, b)]
                c = len(tils_e)
                for ci in range(-(-c // P)):
                    for t in np.unique(tils_e[ci * P:(ci + 1) * P]):
                        own[ci].add(int(t))
            uni_calls.append((g, b, ncols_u * P, ncols_u))
            owners.append([sorted(s) for s in own])

    # tile_prog: per tile, consecutive scol ids in (call, ci) order; same
    # structure for every core. scol blocks padded to multiples of 4.
    raw_prog = [[] for _ in range(T)]      # t -> [(cid, ci)]
    for cid, ((g, b, n16, ncu), own) in enumerate(zip(uni_calls, owners)):
        for ci in range(ncu):
            for t in own[ci]:
                raw_prog[t].append((cid, ci))
    scol_of = {}              # (t, j) -> scol id
    tile_prog = []
    ns = 0
    for t in range(T):
        off = ns
        for j in range(len(raw_prog[t])):
            scol_of[(t, j)] = ns
            ns += 1
        while ns % 4 != 0:
            ns += 1
        tile_prog.append((off, raw_prog[t]))
    NS = ns

    call_slices = []
    off16 = 0
    for (g, b, n16, ncu) in uni_calls:
        call_slices.append((off16, n16 // 16))
        off16 += n16 // 16
    GWtot = off16

    in_maps = []
    layouts = []
    for m in range(M):
        colx = np.full((P, NS), -1.0, dtype=np.float32)
        gidx_parts = []
        for cid, ((g, b, n16, ncu), own) in enumerate(zip(uni_calls,
                                                          owners)):
            rows_e, cols_e, tils_e = per_core_seg[(m, g, b)]
            c = len(rows_e)
            idx = np.zeros(n16, dtype=np.int64)
            idx[:c] = rows_e
            gidx_parts.append(np.tile(
                idx.reshape(n16 // 16, 16).T.astype(np.int16), (8, 1)))
        gidx_w = (np.concatenate(gidx_parts, axis=1) if gidx_parts
                  else np.zeros((P, 16), np.int16))
        # fill colx per tile program entry
        jidx = [0] * T
        for cid, ((g, b, n16, ncu), own) in enumerate(zip(uni_calls,
                                                          owners)):
            rows_e, cols_e, tils_e = per_core_seg[(m, g, b)]
            c = len(rows_e)
            for ci in range(ncu):
                lo, hi = ci * P, min((ci + 1) * P, c)
                for t in own[ci]:
                    sc = np.full(P, -1.0, dtype=np.float32)
                    if hi > lo:
                        seg_t = tils_e[lo:hi]
                        msk = seg_t == t
                        if msk.any():
                            sc[np.arange(lo, hi)[msk] - lo] = \
                                cols_e[lo:hi][msk]
                    colx[:, scol_of[(t, jidx[t])]] = sc
                    jidx[t] += 1

        x_own = np.zeros((SHP, IN), dtype=np.float32)
        deg_own = np.ones(SHP, dtype=np.float32)
        mask_own = np.zeros(SHP, dtype=np.float32)
        sl = slice(m * SH, (m + 1) * SH)
        pos_m = node_pos[sl]
        x_own[pos_m] = x[sl]
        deg_own[pos_m] = deg[sl]
        mask_own[pos_m] = 1.0

        iota4 = np.tile(np.arange(P, dtype=np.float32), (P, 4))

        in_maps.append({
            "xtl": np.ascontiguousarray(
                x_own.reshape(T, P, IN).transpose(1, 0, 2).reshape(P, T * IN)),
            "xT": np.ascontiguousarray(x_own.T.astype(_BF16)),
            "deg": np.ascontiguousarray(deg_own.reshape(T, P).T),
            "mask": np.ascontiguousarray(mask_own.reshape(T, P).T),
            "gidx": np.ascontiguousarray(gidx_w),
            "colx": np.ascontiguousarray(colx),
            "iota4": np.ascontiguousarray(iota4),
            "W": np.ascontiguousarray(W.astype(_BF16)),
            "skipW": np.ascontiguousarray(skip_W.astype(_BF16)),
            "gamma": np.ascontiguousarray(
                gamma.astype(np.float32).reshape(1, OUT)),
            "beta": np.ascontiguousarray(
                beta.astype(np.float32).reshape(1, OUT)),
        })
        layouts.append({"calls": list(uni_calls),
                        "call_slices": call_slices,
                        "tile_prog": tile_prog, "NS": NS,
                        "gidx_cols": gidx_w.shape[1],
                        "colx_cols": colx.shape[1]})
    return in_maps, layouts, SH, T, SHP, NFP, NB, BK, NG, node_pos


def _pad_inmaps(in_maps, layouts):
    """Pad per-core gidx/colx arrays to a common width so one set of dram
    tensor shapes serves all cores (SPMD)."""
    gw = max(l["gidx_cols"] for l in layouts)
    cw = max(l["colx_cols"] for l in layouts)
    cw = -(-cw // 4) * 4
    for im, l in zip(in_maps, layouts):
        g = im["gidx"]
        if g.shape[1] < gw:
            im["gidx"] = np.ascontiguousarray(
                np.concatenate([g, np.zeros((P, gw - g.shape[1]), np.int16)],
                               axis=1))
        c = im["colx"]
        if c.shape[1] < cw:
            im["colx"] = np.ascontiguousarray(
                np.concatenate([c, np.full((P, cw - c.shape[1]), -1.0,
                                           np.float32)], axis=1))
    return gw, cw


def _build(M, N, IN, OUT, T, NB, GT, layouts, GW, CW):
    from concourse import bacc, mybir, tile
    from concourse.masks import make_identity

    dt = mybir.dt
    Alu = mybir.AluOpType
    Act = mybir.ActivationFunctionType

    SHP = T * P
    NFP = M * SHP
    BK = NFP // NB
    IN2 = 2 * IN
    BN_EPS = 1e-5
    NG = -(-T // GT)

    nc = bacc.Bacc("TRN2", target_bir_lowering=False, debug=False,
                   num_devices=M)

    xtl_d = nc.dram_tensor("xtl", [P, T * IN], dt.float32,
                           kind="ExternalInput")
    xT_d = nc.dram_tensor("xT", [IN, SHP], dt.bfloat16, kind="ExternalInput")
    deg_d = nc.dram_tensor("deg", [P, T], dt.float32, kind="ExternalInput")
    mask_d = nc.dram_tensor("mask", [P, T], dt.float32, kind="ExternalInput")
    gidx_d = nc.dram_tensor("gidx", [P, GW], dt.int16, kind="ExternalInput")
    colx_d = nc.dram_tensor("colx", [P, CW], dt.float32,
                            kind="ExternalInput")
    iota4_d = nc.dram_tensor("iota4", [P, 4 * P], dt.float32,
                             kind="ExternalInput")
    W_d = nc.dram_tensor("W", [IN, OUT], dt.bfloat16, kind="ExternalInput")
    skipW_d = nc.dram_tensor("skipW", [IN, OUT], dt.bfloat16,
                             kind="ExternalInput")
    gamma_d = nc.dram_tensor("gamma", [1, OUT], dt.float32,
                             kind="ExternalInput")
    beta_d = nc.dram_tensor("beta", [1, OUT], dt.float32,
                            kind="ExternalInput")
    out_d = nc.dram_tensor("out", [SHP, OUT], dt.float32,
                           kind="ExternalOutput")

    y_local = nc.dram_tensor("y_local", [SHP, IN2], dt.bfloat16)
    y_full = nc.dram_tensor("y_full", [NFP, IN2], dt.bfloat16)
    st_local = nc.dram_tensor("st_local", [1, 2 * OUT], dt.float32)
    st_global = nc.dram_tensor("st_global", [1, 2 * OUT], dt.float32,
                               addr_space="Shared")

    rg = [list(range(M))]
    lay = layouts[0]   # SPMD: all cores share structure of core 0? NO —
    # layouts differ per core; SPMD requires ONE program. We use core 0's
    # structure ONLY where identical; per-core variable parts (call sizes,
    # tile programs) must be THE SAME across cores for a single SPMD
    # program. They are not — so we take the MAX/union approach: pad all
    # cores to core-0's... Instead: require same structure via layout
    # normalization done in kernel() (see _normalize_layouts).

    with tile.TileContext(nc) as tc:
        with (
            tc.tile_pool(name="const", bufs=1) as cpool,
            tc.tile_pool(name="xload", bufs=3) as xpool,
            tc.tile_pool(name="gidxp", bufs=2) as gxpool,
            tc.tile_pool(name="gather", bufs=2) as gpool,
            tc.tile_pool(name="sel", bufs=4) as spool,
            tc.tile_pool(name="evac", bufs=3) as epool,
            tc.tile_pool(name="outt", bufs=3) as opool,
            tc.tile_pool(name="ps_agg", bufs=2, space="PSUM") as ps_agg,
            tc.tile_pool(name="ps_tr", bufs=2, space="PSUM") as ps_tr,
            tc.tile_pool(name="ps_out", bufs=2, space="PSUM") as ps_out,
        ):
            W_sb = cpool.tile([IN, OUT], dt.bfloat16, tag="W")
            nc.sync.dma_start(W_sb[:], W_d[:, :])
            skipW_sb = cpool.tile([IN, OUT], dt.bfloat16, tag="skipW")
            nc.sync.dma_start(skipW_sb[:], skipW_d[:, :])
            iota4_sb = cpool.tile([P, 4, P], dt.float32, tag="iota4")
            nc.sync.dma_start(iota4_sb[:], iota4_d[:, :])
            deg_sb = cpool.tile([P, T], dt.float32, tag="deg")
            nc.sync.dma_start(deg_sb[:], deg_d[:, :])
            mask_sb = cpool.tile([P, T], dt.float32, tag="mask")
            nc.sync.dma_start(mask_sb[:], mask_d[:, :])
            colx_sb = cpool.tile([P, CW], dt.float32, tag="colx")
            nc.sync.dma_start(colx_sb[:], colx_d[:, :])
            xT_sb = cpool.tile([IN, SHP], dt.bfloat16, tag="xT")
            nc.sync.dma_start(xT_sb[:], xT_d[:, :])
            gamma_sb = cpool.tile([1, OUT], dt.float32, tag="gamma")
            nc.sync.dma_start(gamma_sb[:], gamma_d[:, :])
            beta_sb = cpool.tile([1, OUT], dt.float32, tag="beta")
            nc.sync.dma_start(beta_sb[:], beta_d[:, :])

            ident = cpool.tile([P, P], dt.float32, tag="ident")
            make_identity(nc, ident[:])
            identb = cpool.tile([P, P], dt.bfloat16, tag="identb")
            nc.vector.tensor_copy(identb[:], ident[:])
            ones_col = cpool.tile([P, 1], dt.float32, tag="ones_col")
            nc.vector.memset(ones_col[:], 1.0)
            ones_row = cpool.tile([1, P], dt.float32, tag="ones_row")
            nc.vector.memset(ones_row[:], 1.0)

            y_sb = cpool.tile([P, T, IN2], dt.bfloat16, tag="y_sb")
            vbuf = cpool.tile([P, T * OUT], dt.float32, tag="vbuf")
            acc_sum = cpool.tile([P, OUT], dt.float32, tag="acc_sum")
            acc_sq = cpool.tile([P, OUT], dt.float32, tag="acc_sq")

            dinv_sb = cpool.tile([P, T], dt.float32, tag="dinv")
            rec_t = cpool.tile([P, T], dt.float32, tag="rec_t")
            nc.vector.reciprocal(rec_t[:], deg_sb[:])
            nc.scalar.activation(dinv_sb[:], rec_t[:], Act.Sqrt)

            # ---- phase A: y = dinv*x -> bf16 hi/lo in y_sb + DRAM ----
            # x loaded in 14-tile slabs (few big DMAs instead of 98 small)
            BT = 14
            for t0 in range(0, T, BT):
                nb_ = min(BT, T - t0)
                xt_ = xpool.tile([P, BT * IN], dt.float32, tag="xt_")
                nc.sync.dma_start(xt_[:, 0:nb_ * IN],
                                  xtl_d[:, t0 * IN:(t0 + nb_) * IN])
                for k in range(nb_):
                    t = t0 + k
                    y32 = xpool.tile([P, IN], dt.float32, tag="y32")
                    nc.vector.tensor_scalar(
                        y32[:], xt_[:, k * IN:(k + 1) * IN],
                        dinv_sb[:, t:t + 1], None, Alu.mult)
                    nc.vector.tensor_copy(y_sb[:, t, 0:IN], y32[:])
                    nc.vector.tensor_tensor(
                        y_sb[:, t, IN:IN2], y32[:], y_sb[:, t, 0:IN],
                        Alu.subtract)
                    nc.sync.dma_start(y_local[t * P:(t + 1) * P, :],
                                      y_sb[:, t, :])

            nc.gpsimd.collective_compute(
                "AllGather", Alu.bypass, replica_groups=rg,
                ins=[y_local.ap().opt()], outs=[y_full.ap().opt()])

            # ---- main loop over groups ----
            calls = lay["calls"]
            call_slices = lay["call_slices"]
            tile_prog = lay["tile_prog"]
            calls_of_group = [[] for _ in range(NG)]
            for cid, (g, b, n16, ncol) in enumerate(calls):
                calls_of_group[g].append(cid)

            GXW = max(w16 for (_, w16) in call_slices)
            GCOL = max(ncol for (_, _, _, ncol) in calls)
            gbuf = {}    # call_id -> (tile, ncols)
            for g in range(NG):
                for cid in calls_of_group[g]:
                    (gg, b, n16, ncol) = calls[cid]
                    off16, w16 = call_slices[cid]
                    gx = gxpool.tile([P, GXW], dt.int16, tag=f"gx{b}")
                    nc.sync.dma_start(gx[:, 0:w16],
                                      gidx_d[:, off16:off16 + w16])
                    Gb = gpool.tile([P, GCOL, IN2], dt.bfloat16, tag=f"G{b}")
                    nc.gpsimd.dma_gather(
                        Gb[:, 0:ncol, :], y_full[b * BK:(b + 1) * BK, :],
                        gx[:, 0:w16], n16, n16, IN2,
                        single_packet=(n16 <= 1024))
                    gbuf[cid] = (Gb, ncol)

                t0 = g * GT
                for t in range(t0, min(t0 + GT, T)):
                    off, ents = tile_prog[t]
                    ne = len(ents)
                    pagg = ps_agg.tile([P, IN2], dt.float32, tag="pagg")
                    # self-loop: pagg = I^T @ y_tile
                    nc.tensor.matmul(pagg[:], lhsT=identb[:],
                                     rhs=y_sb[:, t, :],
                                     start=True, stop=(ne == 0))
                    # batched x4 one-hot builds: one DVE op per 4 chunks
                    # (colx is 4-aligned per tile; pad columns are all -1)
                    Sts = []
                    for si in range(-(-ne // 4)):
                        S4 = spool.tile([P, 4, P], dt.bfloat16, tag="S4")
                        o4 = off + si * 4
                        cslc = colx_sb[:, o4:o4 + 4]
                        nc.vector.tensor_tensor(
                            S4[:], iota4_sb[:],
                            cslc.unsqueeze(2).broadcast_to((P, 4, P)),
                            Alu.is_equal)
                        Sts.append(S4)
                    for j, (cid, ci) in enumerate(ents):
                        Gb, ncol = gbuf[cid]
                        nc.tensor.matmul(pagg[:],
                                         lhsT=Sts[j // 4][:, j % 4, :],
                                         rhs=Gb[:, ci, :],
                                         start=False, stop=(j == ne - 1))

                    aggs = epool.tile([P, IN], dt.float32, tag="aggs")
                    nc.vector.tensor_copy(aggs[:], pagg[:, 0:IN])
                    nc.vector.tensor_tensor(aggs[:], aggs[:],
                                            pagg[:, IN:IN2], Alu.add)
                    agg = epool.tile([P, IN], dt.float32, tag="agg")
                    nc.vector.tensor_scalar(
                        agg[:], aggs[:], dinv_sb[:, t:t + 1], None, Alu.mult)
                    paggT = ps_tr.tile([IN, P], dt.float32, tag="paggT")
                    nc.tensor.transpose(paggT[:], agg[:], ident[:])
                    aggT = epool.tile([IN, P], dt.bfloat16, tag="aggT")
                    nc.vector.tensor_copy(aggT[:], paggT[:])

                    pout = ps_out.tile([P, OUT], dt.float32, tag="pout")
                    nc.tensor.matmul(pout[:], lhsT=aggT[:], rhs=W_sb[:],
                                     start=True, stop=False)
                    nc.tensor.matmul(pout[:],
                                     lhsT=xT_sb[:, t * P:(t + 1) * P],
                                     rhs=skipW_sb[:], start=False, stop=True)
                    v = vbuf[:, t * OUT:(t + 1) * OUT]
                    nc.vector.tensor_scalar(
                        v, pout[:], mask_sb[:, t:t + 1], None, Alu.mult)
                    sq = epool.tile([P, OUT], dt.float32, tag="sq")
                    nc.vector.tensor_tensor(sq[:], v, v, Alu.mult)
                    if t == 0:
                        nc.vector.tensor_copy(acc_sum[:], v)
                        nc.vector.tensor_copy(acc_sq[:], sq[:])
                    else:
                        nc.vector.tensor_tensor(acc_sum[:], acc_sum[:], v,
                                                Alu.add)
                        nc.vector.tensor_tensor(acc_sq[:], acc_sq[:], sq[:],
                                                Alu.add)

            # ---- BN stats + apply + relu ----
            pst1 = ps_agg.tile([1, OUT], dt.float32, tag="pagg")
            nc.tensor.matmul(pst1[:], lhsT=ones_col[:], rhs=acc_sum[:],
                             start=True, stop=True)
            pst2 = ps_tr.tile([1, OUT], dt.float32, tag="paggT")
            nc.tensor.matmul(pst2[:], lhsT=ones_col[:], rhs=acc_sq[:],
                             start=True, stop=True)
            st_sb = cpool.tile([1, 2 * OUT], dt.float32, tag="st_sb")
            nc.scalar.copy(st_sb[:, 0:OUT], pst1[:])
            nc.scalar.copy(st_sb[:, OUT:2 * OUT], pst2[:])
            nc.sync.dma_start(st_local[:, :], st_sb[:])
            nc.gpsimd.collective_compute(
                "AllReduce", Alu.add, replica_groups=rg,
                ins=[st_local.ap().opt()], outs=[st_global.ap().opt()])
            sg_sb = cpool.tile([1, 2 * OUT], dt.float32, tag="sg_sb")
            nc.sync.dma_start(sg_sb[:], st_global[:, :])

            inv_n = 1.0 / float(N)
            mean_sb = cpool.tile([1, OUT], dt.float32, tag="mean_sb")
            nc.vector.tensor_scalar(mean_sb[:], sg_sb[:, 0:OUT], inv_n, None,
                                    Alu.mult)
            var_sb = cpool.tile([1, OUT], dt.float32, tag="var_sb")
            nc.vector.tensor_scalar(var_sb[:], sg_sb[:, OUT:2 * OUT], inv_n,
                                    None, Alu.mult)
            msq = cpool.tile([1, OUT], dt.float32, tag="msq")
            nc.vector.tensor_tensor(msq[:], mean_sb[:], mean_sb[:], Alu.mult)
            nc.vector.tensor_tensor(var_sb[:], var_sb[:], msq[:],
                                    Alu.subtract)
            nc.vector.tensor_scalar(var_sb[:], var_sb[:], BN_EPS, None,
                                    Alu.add)
            rvar = cpool.tile([1, OUT], dt.float32, tag="rvar")
            nc.vector.reciprocal(rvar[:], var_sb[:])
            rstd = cpool.tile([1, OUT], dt.float32, tag="rstd")
            nc.scalar.activation(rstd[:], rvar[:], Act.Sqrt)

            ab_sb = cpool.tile([1, 2 * OUT], dt.float32, tag="ab_sb")
            nc.vector.tensor_tensor(ab_sb[:, 0:OUT], gamma_sb[:], rstd[:],
                                    Alu.mult)
            ma = cpool.tile([1, OUT], dt.float32, tag="ma")
            nc.vector.tensor_tensor(ma[:], mean_sb[:], ab_sb[:, 0:OUT],
                                    Alu.mult)
            nc.vector.tensor_tensor(ab_sb[:, OUT:2 * OUT], beta_sb[:], ma[:],
                                    Alu.subtract)

            prep = ps_out.tile([P, 2 * OUT], dt.float32, tag="prep")
            nc.tensor.matmul(prep[:], lhsT=ones_row[:], rhs=ab_sb[:],
                             start=True, stop=True)
            a_rep = cpool.tile([P, OUT], dt.float32, tag="a_rep")
            nc.scalar.copy(a_rep[:], prep[:, 0:OUT])
            b_rep = cpool.tile([P, OUT], dt.float32, tag="b_rep")
            nc.scalar.copy(b_rep[:], prep[:, OUT:2 * OUT])

            for t in range(T):
                v = vbuf[:, t * OUT:(t + 1) * OUT]
                o1 = opool.tile([P, OUT], dt.float32, tag="o1")
                nc.vector.tensor_tensor(o1[:], v, a_rep[:], Alu.mult)
                nc.vector.tensor_tensor(o1[:], o1[:], b_rep[:], Alu.add)
                o2 = opool.tile([P, OUT], dt.float32, tag="o2")
                nc.scalar.activation(o2[:], o1[:], Act.Relu)
                nc.sync.dma_start(out_d[t * P:(t + 1) * P, :], o2[:])

    nc.compile()
    return nc


def _normalize_layouts(in_maps, layouts):
    """SPMD needs one program for all cores: pad every core's call list and
    tile programs to core-0-compatible structure. We instead rebuild each
    core's data so that the STRUCTURE (call count/sizes per group, per-tile
    program lengths) equals the element-wise maximum across cores, padding
    with inactive entries (idx=-1 rows / colx=-1 columns)."""
    M = len(layouts)
    NGc = max(max((g for (g, b, n, c) in l["calls"]), default=0)
              for l in layouts) + 1
    # unify call structure per (g, b): nidx16/ncols = max over cores
    sizes = {}
    for l in layouts:
        for (g, b, n16, nc_) in l["calls"]:
            k = (g, b)
            n0, c0 = sizes.get(k, (0, 0))
            sizes[k] = (max(n0, n16), max(c0, nc_))
    # per-tile program length = max over cores
    T = len(layouts[0]["tile_prog"])
    plen = [0] * T
    for l in layouts:
        for t in range(T):
            plen[t] = max(plen[t], len(l["tile_prog"][t][1]))
    return sizes, plen, NGc


def kernel(x, edge_index, W, bias, skip_W, gamma, beta, _trace=False,
           _return_results=False):
    x = np.asarray(x, dtype=np.float32)
    edge_index = np.asarray(edge_index, dtype=np.int32)
    M = 8
    N, IN = x.shape
    OUT = np.asarray(W).shape[1]
    GT = 7

    in_maps, layouts, SH, T, SHP, NFP, NB, BK, NG, node_pos = _host_prep(
        x, edge_index, W, skip_W, gamma, beta, M, IN, OUT, GT)

    # structure is union-built in _host_prep: identical across cores
    GW, CW = _pad_inmaps(in_maps, layouts)
    for im in in_maps:
        assert im["gidx"].shape[1] == GW and im["colx"].shape[1] == CW

    structure = tuple(layouts[0]["calls"]) + tuple(
        (o, len(e)) for o, e in layouts[0]["tile_prog"])
    key = (M, N, IN, OUT, T, NB, GT, GW, CW, hash(structure))
    if key not in _KCACHE:
        _KCACHE[key] = _build(M, N, IN, OUT, T, NB, GT, layouts, GW, CW)
    nc = _KCACHE[key]

    from concourse import bass_utils
    res = bass_utils.run_bass_kernel_spmd(
        nc, in_maps, core_ids=list(range(M)), trace=_trace)
    outs = [res.results[m]["out"][node_pos[m * SH:(m + 1) * SH]]
            for m in range(M)]
    full = np.concatenate(outs, axis=0).astype(np.float32)
    if _return_results:
        return full, res
    return full


# revision 11
# speedup vs baseline: 1.1148x; 1.1148x over previous
"""GCN layer (GCNConv + skip + BN + ReLU) on 8 TRN2 cores — v3.

vs baseline: self-loops removed from the gather (added via one identity
matmul per tile); edges packed per (tile-group, bank) with no per-tile
padding (boundary gather-columns get two masked S-columns) -> 200.6k gather
descriptors/core vs 250.9k; S one-hots built on DVE (fp32-in bf16-out,
batched x4) instead of 2 ACT ops per chunk; transform matmuls in bf16.
"""

import numpy as np
import ml_dtypes

P = 128
BANK_MAX = 32768

_BF16 = ml_dtypes.bfloat16

_KCACHE = {}


def _host_prep(x, edge_index, W, skip_W, gamma, beta, M, IN, OUT, GT):
    N = x.shape[0]
    SH = N // M
    T = -(-SH // P)
    SHP = T * P
    NFP = M * SHP
    NB = -(-NFP // BANK_MAX)
    BK = NFP // NB
    NG = -(-T // GT)
    assert NFP % NB == 0 and BK <= BANK_MAX

    row = edge_index[0].astype(np.int64)
    col = edge_index[1].astype(np.int64)
    loops = np.arange(N, dtype=np.int64)
    deg = np.bincount(np.concatenate([col, loops]),
                      minlength=N).astype(np.float32)

    # snake-balanced node -> (tile, slot)
    node_pos = np.empty(N, dtype=np.int64)
    for m in range(M):
        dg = deg[m * SH:(m + 1) * SH]
        order_n = np.argsort(-dg, kind="stable")
        ranks = np.empty(SH, dtype=np.int64)
        ranks[order_n] = np.arange(SH)
        rounds = ranks // T
        tpos = ranks % T
        tile_of = np.where(rounds % 2 == 0, tpos, T - 1 - tpos)
        node_pos[m * SH:(m + 1) * SH] = tile_of * P + rounds

    # half-split padded-global layout: half h of every core's shard is
    # AllGathered separately, so banks 0..NB/2-1 depend only on AG1
    HSHP = SHP // 2
    posr = node_pos[row]
    hlf = posr // HSHP
    src_pad = hlf * (M * HSHP) + (row // SH) * HSHP + (posr - hlf * HSHP)
    bank = src_pad // BK
    core = col // SH
    post = node_pos[col]
    tile = post // P
    grp = (core * NG + tile // GT) * NB + bank
    key = grp * T + tile
    order = np.argsort(key, kind="stable")
    srcrow_s = (src_pad - bank * BK)[order]
    colloc_s = (post % P)[order]
    tile_s = tile[order]
    grp_s = grp[order]

    NGRP = M * NG * NB
    cnts = np.bincount(grp_s, minlength=NGRP)
    starts = np.zeros(NGRP + 1, dtype=np.int64)
    np.cumsum(cnts, out=starts[1:])

    # ---- UNION structure: identical program for every core ----
    # per (g, b): ncols_u = max over cores; per column: union of owner tiles
    uni_calls = []            # (g, b, nidx16, ncols)
    owners = []               # per call: list per ci of sorted owner tiles
    per_core_seg = {}         # (m, g, b) -> (rows_e, cols_e, tils_e)
    for g in range(NG):
        for b in range(NB):
            ncols_u = 0
            own = None
            for m in range(M):
                gi = (m * NG + g) * NB + b
                c = int(cnts[gi])
                s0 = starts[gi]
                per_core_seg[(m, g, b)] = (
                    srcrow_s[s0:s0 + c], colloc_s[s0:s0 + c],
                    tile_s[s0:s0 + c])
                ncols_u = max(ncols_u, -(-c // P))
            if ncols_u == 0:
                continue
            own = [set() for _ in range(ncols_u)]
            for m in range(M):
                (_, _, tils_e) = per_core_seg[(m, g, b)]
                c = len(tils_e)
                for ci in range(-(-c // P)):
                    for t in np.unique(tils_e[ci * P:(ci + 1) * P]):
                        own[ci].add(int(t))
            uni_calls.append((g, b, ncols_u * P, ncols_u))
            owners.append([sorted(s) for s in own])

    # tile_prog: per tile, consecutive scol ids in (call, ci) order; same
    # structure for every core. scol blocks padded to multiples of 4.
    raw_prog = [[] for _ in range(T)]      # t -> [(cid, ci)]
    for cid, ((g, b, n16, ncu), own) in enumerate(zip(uni_calls, owners)):
        for ci in range(ncu):
            for t in own[ci]:
                raw_prog[t].append((cid, ci))
    scol_of = {}              # (t, j) -> scol id
    tile_prog = []
    ns = 0
    for t in range(T):
        off = ns
        for j in range(len(raw_prog[t])):
            scol_of[(t, j)] = ns
            ns += 1
        while ns % 4 != 0:
            ns += 1
        tile_prog.append((off, raw_prog[t]))
    NS = ns

    call_slices = []
    off16 = 0
    for (g, b, n16, ncu) in uni_calls:
        call_slices.append((off16, n16 // 16))
        off16 += n16 // 16
    GWtot = off16

    in_maps = []
    layouts = []
    for m in range(M):
        colx = np.full((P, NS), -1.0, dtype=np.float32)
        gidx_parts = []
        for cid, ((g, b, n16, ncu), own) in enumerate(zip(uni_calls,
                                                          owners)):
            rows_e, cols_e, tils_e = per_core_seg[(m, g, b)]
            c = len(rows_e)
            idx = np.zeros(n16, dtype=np.int64)
            idx[:c] = rows_e
            gidx_parts.append(np.tile(
                idx.reshape(n16 // 16, 16).T.astype(np.int16), (8, 1)))
        gidx_w = (np.concatenate(gidx_parts, axis=1) if gidx_parts
                  else np.zeros((P, 16), np.int16))
        # fill colx per tile program entry
        jidx = [0] * T
        for cid, ((g, b, n16, ncu), own) in enumerate(zip(uni_calls,
                                                          owners)):
            rows_e, cols_e, tils_e = per_core_seg[(m, g, b)]
            c = len(rows_e)
            for ci in range(ncu):
                lo, hi = ci * P, min((ci + 1) * P, c)
                for t in own[ci]:
                    sc = np.full(P, -1.0, dtype=np.float32)
                    if hi > lo:
                        seg_t = tils_e[lo:hi]
                        msk = seg_t == t
                        if msk.any():
                            sc[np.arange(lo, hi)[msk] - lo] = \
                                cols_e[lo:hi][msk]
                    colx[:, scol_of[(t, jidx[t])]] = sc
                    jidx[t] += 1

        x_own = np.zeros((SHP, IN), dtype=np.float32)
        deg_own = np.ones(SHP, dtype=np.float32)
        mask_own = np.zeros(SHP, dtype=np.float32)
        sl = slice(m * SH, (m + 1) * SH)
        pos_m = node_pos[sl]
        x_own[pos_m] = x[sl]
        deg_own[pos_m] = deg[sl]
        mask_own[pos_m] = 1.0

        iota4 = np.tile(np.arange(P, dtype=np.float32), (P, 4))

        in_maps.append({
            "xtl": np.ascontiguousarray(
                x_own.reshape(T, P, IN).transpose(1, 0, 2).reshape(P, T * IN)),
            "xT": np.ascontiguousarray(x_own.T.astype(_BF16)),
            "deg": np.ascontiguousarray(deg_own.reshape(T, P).T),
            "mask": np.ascontiguousarray(mask_own.reshape(T, P).T),
            "gidx": np.ascontiguousarray(gidx_w),
            "colx": np.ascontiguousarray(colx),
            "iota4": np.ascontiguousarray(iota4),
            "W": np.ascontiguousarray(W.astype(_BF16)),
            "skipW": np.ascontiguousarray(skip_W.astype(_BF16)),
            "gamma": np.ascontiguousarray(
                gamma.astype(np.float32).reshape(1, OUT)),
            "beta": np.ascontiguousarray(
                beta.astype(np.float32).reshape(1, OUT)),
        })
        layouts.append({"calls": list(uni_calls),
                        "call_slices": call_slices,
                        "tile_prog": tile_prog, "NS": NS,
                        "gidx_cols": gidx_w.shape[1],
                        "colx_cols": colx.shape[1]})
    return in_maps, layouts, SH, T, SHP, NFP, NB, BK, NG, node_pos


def _pad_inmaps(in_maps, layouts):
    """Pad per-core gidx/colx arrays to a common width so one set of dram
    tensor shapes serves all cores (SPMD)."""
    gw = max(l["gidx_cols"] for l in layouts)
    cw = max(l["colx_cols"] for l in layouts)
    cw = -(-cw // 4) * 4
    for im, l in zip(in_maps, layouts):
        g = im["gidx"]
        if g.shape[1] < gw:
            im["gidx"] = np.ascontiguousarray(
                np.concatenate([g, np.zeros((P, gw - g.shape[1]), np.int16)],
                               axis=1))
        c = im["colx"]
        if c.shape[1] < cw:
            im["colx"] = np.ascontiguousarray(
                np.concatenate([c, np.full((P, cw - c.shape[1]), -1.0,
                                           np.float32)], axis=1))
    return gw, cw


def _build(M, N, IN, OUT, T, NB, GT, layouts, GW, CW):
    from concourse import bacc, mybir, tile
    from concourse.masks import make_identity

    dt = mybir.dt
    Alu = mybir.AluOpType
    Act = mybir.ActivationFunctionType

    SHP = T * P
    NFP = M * SHP
    BK = NFP // NB
    IN2 = 2 * IN
    BN_EPS = 1e-5
    NG = -(-T // GT)

    nc = bacc.Bacc("TRN2", target_bir_lowering=False, debug=False,
                   num_devices=M)

    xtl_d = nc.dram_tensor("xtl", [P, T * IN], dt.float32,
                           kind="ExternalInput")
    xT_d = nc.dram_tensor("xT", [IN, SHP], dt.bfloat16, kind="ExternalInput")
    deg_d = nc.dram_tensor("deg", [P, T], dt.float32, kind="ExternalInput")
    mask_d = nc.dram_tensor("mask", [P, T], dt.float32, kind="ExternalInput")
    gidx_d = nc.dram_tensor("gidx", [P, GW], dt.int16, kind="ExternalInput")
    colx_d = nc.dram_tensor("colx", [P, CW], dt.float32,
                            kind="ExternalInput")
    iota4_d = nc.dram_tensor("iota4", [P, 4 * P], dt.float32,
                             kind="ExternalInput")
    W_d = nc.dram_tensor("W", [IN, OUT], dt.bfloat16, kind="ExternalInput")
    skipW_d = nc.dram_tensor("skipW", [IN, OUT], dt.bfloat16,
                             kind="ExternalInput")
    gamma_d = nc.dram_tensor("gamma", [1, OUT], dt.float32,
                             kind="ExternalInput")
    beta_d = nc.dram_tensor("beta", [1, OUT], dt.float32,
                            kind="ExternalInput")
    out_d = nc.dram_tensor("out", [SHP, OUT], dt.float32,
                           kind="ExternalOutput")

    y_local = nc.dram_tensor("y_local", [SHP, IN2], dt.bfloat16)
    y_full = nc.dram_tensor("y_full", [NFP, IN2], dt.bfloat16)
    st_local = nc.dram_tensor("st_local", [1, 2 * OUT], dt.float32)
    st_global = nc.dram_tensor("st_global", [1, 2 * OUT], dt.float32,
                               addr_space="Shared")

    rg = [list(range(M))]
    lay = layouts[0]   # SPMD: all cores share structure of core 0? NO —
    # layouts differ per core; SPMD requires ONE program. We use core 0's
    # structure ONLY where identical; per-core variable parts (call sizes,
    # tile programs) must be THE SAME across cores for a single SPMD
    # program. They are not — so we take the MAX/union approach: pad all
    # cores to core-0's... Instead: require same structure via layout
    # normalization done in kernel() (see _normalize_layouts).

    with tile.TileContext(nc) as tc:
        with (
            tc.tile_pool(name="const", bufs=1) as cpool,
            tc.tile_pool(name="xload", bufs=3) as xpool,
            tc.tile_pool(name="gidxp", bufs=2) as gxpool,
            tc.tile_pool(name="gather", bufs=2) as gpool,
            tc.tile_pool(name="sel", bufs=4) as spool,
            tc.tile_pool(name="evac", bufs=3) as epool,
            tc.tile_pool(name="outt", bufs=3) as opool,
            tc.tile_pool(name="ps_agg", bufs=2, space="PSUM") as ps_agg,
            tc.tile_pool(name="ps_tr", bufs=2, space="PSUM") as ps_tr,
            tc.tile_pool(name="ps_out", bufs=2, space="PSUM") as ps_out,
        ):
            W_sb = cpool.tile([IN, OUT], dt.bfloat16, tag="W")
            nc.sync.dma_start(W_sb[:], W_d[:, :])
            skipW_sb = cpool.tile([IN, OUT], dt.bfloat16, tag="skipW")
            nc.sync.dma_start(skipW_sb[:], skipW_d[:, :])
            iota4_sb = cpool.tile([P, 4, P], dt.float32, tag="iota4")
            nc.sync.dma_start(iota4_sb[:], iota4_d[:, :])
            deg_sb = cpool.tile([P, T], dt.float32, tag="deg")
            nc.sync.dma_start(deg_sb[:], deg_d[:, :])
            mask_sb = cpool.tile([P, T], dt.float32, tag="mask")
            nc.sync.dma_start(mask_sb[:], mask_d[:, :])
            colx_sb = cpool.tile([P, CW], dt.float32, tag="colx")
            nc.sync.dma_start(colx_sb[:], colx_d[:, :])
            xT_sb = cpool.tile([IN, SHP], dt.bfloat16, tag="xT")
            nc.sync.dma_start(xT_sb[:], xT_d[:, :])
            gamma_sb = cpool.tile([1, OUT], dt.float32, tag="gamma")
            nc.sync.dma_start(gamma_sb[:], gamma_d[:, :])
            beta_sb = cpool.tile([1, OUT], dt.float32, tag="beta")
            nc.sync.dma_start(beta_sb[:], beta_d[:, :])

            ident = cpool.tile([P, P], dt.float32, tag="ident")
            make_identity(nc, ident[:])
            identb = cpool.tile([P, P], dt.bfloat16, tag="identb")
            nc.vector.tensor_copy(identb[:], ident[:])
            ones_col = cpool.tile([P, 1], dt.float32, tag="ones_col")
            nc.vector.memset(ones_col[:], 1.0)
            ones_row = cpool.tile([1, P], dt.float32, tag="ones_row")
            nc.vector.memset(ones_row[:], 1.0)

            y_sb = cpool.tile([P, T, IN2], dt.bfloat16, tag="y_sb")
            vbuf = cpool.tile([P, T * OUT], dt.float32, tag="vbuf")
            acc_sum = cpool.tile([P, OUT], dt.float32, tag="acc_sum")
            acc_sq = cpool.tile([P, OUT], dt.float32, tag="acc_sq")

            dinv_sb = cpool.tile([P, T], dt.float32, tag="dinv")
            rec_t = cpool.tile([P, T], dt.float32, tag="rec_t")
            nc.vector.reciprocal(rec_t[:], deg_sb[:])
            nc.scalar.activation(dinv_sb[:], rec_t[:], Act.Sqrt)

            # ---- phase A: y = dinv*x -> bf16 hi/lo in y_sb + DRAM ----
            # x loaded in 14-tile slabs; y AllGathered in two halves so
            # gathers on the first half's banks start while AG2 flies
            BT = 14
            TH = T // 2
            HS = TH * P
            HF = M * HS

            def phase_a(t_lo, t_hi):
                for t0 in range(t_lo, t_hi, BT):
                    nb_ = min(BT, t_hi - t0)
                    xt_ = xpool.tile([P, BT * IN], dt.float32, tag="xt_")
                    nc.sync.dma_start(xt_[:, 0:nb_ * IN],
                                      xtl_d[:, t0 * IN:(t0 + nb_) * IN])
                    for k in range(nb_):
                        t = t0 + k
                        y32 = xpool.tile([P, IN], dt.float32, tag="y32")
                        nc.vector.tensor_scalar(
                            y32[:], xt_[:, k * IN:(k + 1) * IN],
                            dinv_sb[:, t:t + 1], None, Alu.mult)
                        nc.vector.tensor_copy(y_sb[:, t, 0:IN], y32[:])
                        nc.vector.tensor_tensor(
                            y_sb[:, t, IN:IN2], y32[:], y_sb[:, t, 0:IN],
                            Alu.subtract)
                        nc.sync.dma_start(y_local[t * P:(t + 1) * P, :],
                                          y_sb[:, t, :])

            phase_a(0, TH)
            nc.gpsimd.collective_compute(
                "AllGather", Alu.bypass, replica_groups=rg,
                ins=[y_local[0:HS, :].opt()],
                outs=[y_full[0:HF, :].opt()])
            phase_a(TH, T)
            nc.gpsimd.collective_compute(
                "AllGather", Alu.bypass, replica_groups=rg,
                ins=[y_local[HS:SHP, :].opt()],
                outs=[y_full[HF:NFP, :].opt()])

            # ---- main loop over groups ----
            calls = lay["calls"]
            call_slices = lay["call_slices"]
            tile_prog = lay["tile_prog"]
            calls_of_group = [[] for _ in range(NG)]
            for cid, (g, b, n16, ncol) in enumerate(calls):
                calls_of_group[g].append(cid)

            GXW = max(w16 for (_, w16) in call_slices)
            GCOL = max(ncol for (_, _, _, ncol) in calls)
            gbuf = {}    # call_id -> (tile, ncols)
            for g in range(NG):
                for cid in calls_of_group[g]:
                    (gg, b, n16, ncol) = calls[cid]
                    off16, w16 = call_slices[cid]
                    gx = gxpool.tile([P, GXW], dt.int16, tag=f"gx{b}")
                    nc.sync.dma_start(gx[:, 0:w16],
                                      gidx_d[:, off16:off16 + w16])
                    Gb = gpool.tile([P, GCOL, IN2], dt.bfloat16, tag=f"G{b}")
                    nc.gpsimd.dma_gather(
                        Gb[:, 0:ncol, :], y_full[b * BK:(b + 1) * BK, :],
                        gx[:, 0:w16], n16, n16, IN2,
                        single_packet=(n16 <= 1024))
                    gbuf[cid] = (Gb, ncol)

                t0 = g * GT
                for t in range(t0, min(t0 + GT, T)):
                    off, ents = tile_prog[t]
                    ne = len(ents)
                    pagg = ps_agg.tile([P, IN2], dt.float32, tag="pagg")
                    # self-loop: pagg = I^T @ y_tile
                    nc.tensor.matmul(pagg[:], lhsT=identb[:],
                                     rhs=y_sb[:, t, :],
                                     start=True, stop=(ne == 0))
                    # batched x4 one-hot builds: one DVE op per 4 chunks
                    # (colx is 4-aligned per tile; pad columns are all -1)
                    Sts = []
                    for si in range(-(-ne // 4)):
                        S4 = spool.tile([P, 4, P], dt.bfloat16, tag="S4")
                        o4 = off + si * 4
                        cslc = colx_sb[:, o4:o4 + 4]
                        nc.vector.tensor_tensor(
                            S4[:], iota4_sb[:],
                            cslc.unsqueeze(2).broadcast_to((P, 4, P)),
                            Alu.is_equal)
                        Sts.append(S4)
                    for j, (cid, ci) in enumerate(ents):
                        Gb, ncol = gbuf[cid]
                        nc.tensor.matmul(pagg[:],
                                         lhsT=Sts[j // 4][:, j % 4, :],
                                         rhs=Gb[:, ci, :],
                                         start=False, stop=(j == ne - 1))

                    aggs = epool.tile([P, IN], dt.float32, tag="aggs")
                    nc.vector.tensor_copy(aggs[:], pagg[:, 0:IN])
                    nc.vector.tensor_tensor(aggs[:], aggs[:],
                                            pagg[:, IN:IN2], Alu.add)
                    agg = epool.tile([P, IN], dt.float32, tag="agg")
                    nc.vector.tensor_scalar(
                        agg[:], aggs[:], dinv_sb[:, t:t + 1], None, Alu.mult)
                    paggT = ps_tr.tile([IN, P], dt.float32, tag="paggT")
                    nc.tensor.transpose(paggT[:], agg[:], ident[:])
                    aggT = epool.tile([IN, P], dt.bfloat16, tag="aggT")
                    nc.vector.tensor_copy(aggT[:], paggT[:])

                    pout = ps_out.tile([P, OUT], dt.float32, tag="pout")
                    nc.tensor.matmul(pout[:], lhsT=aggT[:], rhs=W_sb[:],
                                     start=True, stop=False)
                    nc.tensor.matmul(pout[:],
                                     lhsT=xT_sb[:, t * P:(t + 1) * P],
                                     rhs=skipW_sb[:], start=False, stop=True)
                    v = vbuf[:, t * OUT:(t + 1) * OUT]
                    nc.vector.tensor_scalar(
                        v, pout[:], mask_sb[:, t:t + 1], None, Alu.mult)
                    sq = epool.tile([P, OUT], dt.float32, tag="sq")
                    nc.vector.tensor_tensor(sq[:], v, v, Alu.mult)
                    if t == 0:
                        nc.vector.tensor_copy(acc_sum[:], v)
                        nc.vector.tensor_copy(acc_sq[:], sq[:])
                    else:
                        nc.vector.tensor_tensor(acc_sum[:], acc_sum[:], v,
                                                Alu.add)
                        nc.vector.tensor_tensor(acc_sq[:], acc_sq[:], sq[:],
                                                Alu.add)

            # ---- BN stats + apply + relu ----
            pst1 = ps_agg.tile([1, OUT], dt.float32, tag="pagg")
            nc.tensor.matmul(pst1[:], lhsT=ones_col[:], rhs=acc_sum[:],
                             start=True, stop=True)
            pst2 = ps_tr.tile([1, OUT], dt.float32, tag="paggT")
            nc.tensor.matmul(pst2[:], lhsT=ones_col[:], rhs=acc_sq[:],
                             start=True, stop=True)
            st_sb = cpool.tile([1, 2 * OUT], dt.float32, tag="st_sb")
            nc.scalar.copy(st_sb[:, 0:OUT], pst1[:])
            nc.scalar.copy(st_sb[:, OUT:2 * OUT], pst2[:])
            nc.sync.dma_start(st_local[:, :], st_sb[:])
            nc.gpsimd.collective_compute(
                "AllReduce", Alu.add, replica_groups=rg,
                ins=[st_local.ap().opt()], outs=[st_global.ap().opt()])
            sg_sb = cpool.tile([1, 2 * OUT], dt.float32, tag="sg_sb")
            nc.sync.dma_start(sg_sb[:], st_global[:, :])

            inv_n = 1.0 / float(N)
            mean_sb = cpool.tile([1, OUT], dt.float32, tag="mean_sb")
            nc.vector.tensor_scalar(mean_sb[:], sg_sb[:, 0:OUT], inv_n, None,
                                    Alu.mult)
            var_sb = cpool.tile([1, OUT], dt.float32, tag="var_sb")
            nc.vector.tensor_scalar(var_sb[:], sg_sb[:, OUT:2 * OUT], inv_n,
                                    None, Alu.mult)
            msq = cpool.tile([1, OUT], dt.float32, tag="msq")
            nc.vector.tensor_tensor(msq[:], mean_sb[:], mean_sb[:], Alu.mult)
            nc.vector.tensor_tensor(var_sb[:], var_sb[:], msq[:],
                                    Alu.subtract)
            nc.vector.tensor_scalar(var_sb[:], var_sb[:], BN_EPS, None,
                                    Alu.add)
            rvar = cpool.tile([1, OUT], dt.float32, tag="rvar")
            nc.vector.reciprocal(rvar[:], var_sb[:])
            rstd = cpool.tile([1, OUT], dt.float32, tag="rstd")
            nc.scalar.activation(rstd[:], rvar[:], Act.Sqrt)

            ab_sb = cpool.tile([1, 2 * OUT], dt.float32, tag="ab_sb")
            nc.vector.tensor_tensor(ab_sb[:, 0:OUT], gamma_sb[:], rstd[:],
                                    Alu.mult)
            ma = cpool.tile([1, OUT], dt.float32, tag="ma")
            nc.vector.tensor_tensor(ma[:], mean_sb[:], ab_sb[:, 0:OUT],
                                    Alu.mult)
            nc.vector.tensor_tensor(ab_sb[:, OUT:2 * OUT], beta_sb[:], ma[:],
                                    Alu.subtract)

            prep = ps_out.tile([P, 2 * OUT], dt.float32, tag="prep")
            nc.tensor.matmul(prep[:], lhsT=ones_row[:], rhs=ab_sb[:],
                             start=True, stop=True)
            a_rep = cpool.tile([P, OUT], dt.float32, tag="a_rep")
            nc.scalar.copy(a_rep[:], prep[:, 0:OUT])
            b_rep = cpool.tile([P, OUT], dt.float32, tag="b_rep")
            nc.scalar.copy(b_rep[:], prep[:, OUT:2 * OUT])

            for t in range(T):
                v = vbuf[:, t * OUT:(t + 1) * OUT]
                o1 = opool.tile([P, OUT], dt.float32, tag="o1")
                nc.vector.tensor_tensor(o1[:], v, a_rep[:], Alu.mult)
                nc.vector.tensor_tensor(o1[:], o1[:], b_rep[:], Alu.add)
                o2 = opool.tile([P, OUT], dt.float32, tag="o2")
                nc.scalar.activation(o2[:], o1[:], Act.Relu)
                nc.sync.dma_start(out_d[t * P:(t + 1) * P, :], o2[:])

    nc.compile()
    return nc


def _normalize_layouts(in_maps, layouts):
    """SPMD needs one program for all cores: pad every core's call list and
    tile programs to core-0-compatible structure. We instead rebuild each
    core's data so that the STRUCTURE (call count/sizes per group, per-tile
    program lengths) equals the element-wise maximum across cores, padding
    with inactive entries (idx=-1 rows / colx=-1 columns)."""
    M = len(layouts)
    NGc = max(max((g for (g, b, n, c) in l["calls"]), default=0)
              for l in layouts) + 1
    # unify call structure per (g, b): nidx16/ncols = max over cores
    sizes = {}
    for l in layouts:
        for (g, b, n16, nc_) in l["calls"]:
            k = (g, b)
            n0, c0 = sizes.get(k, (0, 0))
            sizes[k] = (max(n0, n16), max(c0, nc_))
    # per-tile program length = max over cores
    T = len(layouts[0]["tile_prog"])
    plen = [0] * T
    for l in layouts:
        for t in range(T):
            plen[t] = max(plen[t], len(l["tile_prog"][t][1]))
    return sizes, plen, NGc


def kernel(x, edge_index, W, bias, skip_W, gamma, beta, _trace=False,
           _return_results=False):
    x = np.asarray(x, dtype=np.float32)
    edge_index = np.asarray(edge_index, dtype=np.int32)
    M = 8
    N, IN = x.shape
    OUT = np.asarray(W).shape[1]
    GT = 5

    in_maps, layouts, SH, T, SHP, NFP, NB, BK, NG, node_pos = _host_prep(
        x, edge_index, W, skip_W, gamma, beta, M, IN, OUT, GT)

    # structure is union-built in _host_prep: identical across cores
    GW, CW = _pad_inmaps(in_maps, layouts)
    for im in in_maps:
        assert im["gidx"].shape[1] == GW and im["colx"].shape[1] == CW

    structure = tuple(layouts[0]["calls"]) + tuple(
        (o, len(e)) for o, e in layouts[0]["tile_prog"])
    key = (M, N, IN, OUT, T, NB, GT, GW, CW, hash(structure))
    if key not in _KCACHE:
        _KCACHE[key] = _build(M, N, IN, OUT, T, NB, GT, layouts, GW, CW)
    nc = _KCACHE[key]

    from concourse import bass_utils
    res = bass_utils.run_bass_kernel_spmd(
        nc, in_maps, core_ids=list(range(M)), trace=_trace)
    outs = [res.results[m]["out"][node_pos[m * SH:(m + 1) * SH]]
            for m in range(M)]
    full = np.concatenate(outs, axis=0).astype(np.float32)
    if _return_results:
        return full, res
    return full


# revision 15
# speedup vs baseline: 1.1658x; 1.0457x over previous
"""GCN layer (GCNConv + skip + BN + ReLU) on 8 TRN2 cores — v3.

vs baseline: self-loops removed from the gather (added via one identity
matmul per tile); edges packed per (tile-group, bank) with no per-tile
padding (boundary gather-columns get two masked S-columns) -> 200.6k gather
descriptors/core vs 250.9k; S one-hots built on DVE (fp32-in bf16-out,
batched x4) instead of 2 ACT ops per chunk; transform matmuls in bf16.
"""

import numpy as np
import ml_dtypes

P = 128
BANK_MAX = 32768

_BF16 = ml_dtypes.bfloat16

_KCACHE = {}


def _host_prep(x, edge_index, W, skip_W, gamma, beta, M, IN, OUT, GT):
    N = x.shape[0]
    SH = N // M
    T = -(-SH // P)
    SHP = T * P
    NFP = M * SHP
    NB = -(-NFP // BANK_MAX)
    BK = NFP // NB
    NG = -(-T // GT)
    assert NFP % NB == 0 and BK <= BANK_MAX

    row = edge_index[0].astype(np.int64)
    col = edge_index[1].astype(np.int64)
    loops = np.arange(N, dtype=np.int64)
    deg = np.bincount(np.concatenate([col, loops]),
                      minlength=N).astype(np.float32)

    # snake-balanced node -> (tile, slot)
    node_pos = np.empty(N, dtype=np.int64)
    for m in range(M):
        dg = deg[m * SH:(m + 1) * SH]
        order_n = np.argsort(-dg, kind="stable")
        ranks = np.empty(SH, dtype=np.int64)
        ranks[order_n] = np.arange(SH)
        rounds = ranks // T
        tpos = ranks % T
        tile_of = np.where(rounds % 2 == 0, tpos, T - 1 - tpos)
        node_pos[m * SH:(m + 1) * SH] = tile_of * P + rounds

    # half-split padded-global layout: half h of every core's shard is
    # AllGathered separately, so banks 0..NB/2-1 depend only on AG1
    HSHP = SHP // 2
    posr = node_pos[row]
    hlf = posr // HSHP
    src_pad = hlf * (M * HSHP) + (row // SH) * HSHP + (posr - hlf * HSHP)
    bank = src_pad // BK
    core = col // SH
    post = node_pos[col]
    tile = post // P
    grp = (core * NG + tile // GT) * NB + bank
    key = grp * T + tile
    order = np.argsort(key, kind="stable")
    srcrow_s = (src_pad - bank * BK)[order]
    colloc_s = (post % P)[order]
    tile_s = tile[order]
    grp_s = grp[order]

    NGRP = M * NG * NB
    cnts = np.bincount(grp_s, minlength=NGRP)
    starts = np.zeros(NGRP + 1, dtype=np.int64)
    np.cumsum(cnts, out=starts[1:])

    # ---- UNION structure: identical program for every core ----
    # per (g, b): ncols_u = max over cores; per column: union of owner tiles
    uni_calls = []            # (g, b, nidx16, ncols)
    owners = []               # per call: list per ci of sorted owner tiles
    per_core_seg = {}         # (m, g, b) -> (rows_e, cols_e, tils_e)
    for g in range(NG):
        for b in range(NB):
            ncols_u = 0
            own = None
            for m in range(M):
                gi = (m * NG + g) * NB + b
                c = int(cnts[gi])
                s0 = starts[gi]
                per_core_seg[(m, g, b)] = (
                    srcrow_s[s0:s0 + c], colloc_s[s0:s0 + c],
                    tile_s[s0:s0 + c])
                ncols_u = max(ncols_u, -(-c // P))
            if ncols_u == 0:
                continue
            own = [set() for _ in range(ncols_u)]
            for m in range(M):
                (_, _, tils_e) = per_core_seg[(m, g, b)]
                c = len(tils_e)
                for ci in range(-(-c // P)):
                    for t in np.unique(tils_e[ci * P:(ci + 1) * P]):
                        own[ci].add(int(t))
            uni_calls.append((g, b, ncols_u * P, ncols_u))
            owners.append([sorted(s) for s in own])

    # tile_prog: per tile, consecutive scol ids in (call, ci) order; same
    # structure for every core. scol blocks padded to multiples of 4.
    raw_prog = [[] for _ in range(T)]      # t -> [(cid, ci)]
    for cid, ((g, b, n16, ncu), own) in enumerate(zip(uni_calls, owners)):
        for ci in range(ncu):
            for t in own[ci]:
                raw_prog[t].append((cid, ci))
    scol_of = {}              # (t, j) -> scol id
    tile_prog = []
    ns = 0
    for t in range(T):
        off = ns
        for j in range(len(raw_prog[t])):
            scol_of[(t, j)] = ns
            ns += 1
        while ns % 4 != 0:
            ns += 1
        tile_prog.append((off, raw_prog[t]))
    NS = ns

    call_slices = []
    off16 = 0
    for (g, b, n16, ncu) in uni_calls:
        call_slices.append((off16, n16 // 16))
        off16 += n16 // 16
    GWtot = off16

    in_maps = []
    layouts = []
    for m in range(M):
        colx = np.full((P, NS), -1.0, dtype=np.float32)
        gidx_parts = []
        for cid, ((g, b, n16, ncu), own) in enumerate(zip(uni_calls,
                                                          owners)):
            rows_e, cols_e, tils_e = per_core_seg[(m, g, b)]
            c = len(rows_e)
            idx = np.zeros(n16, dtype=np.int64)
            idx[:c] = rows_e
            gidx_parts.append(np.tile(
                idx.reshape(n16 // 16, 16).T.astype(np.int16), (8, 1)))
        gidx_w = (np.concatenate(gidx_parts, axis=1) if gidx_parts
                  else np.zeros((P, 16), np.int16))
        # fill colx per tile program entry
        jidx = [0] * T
        for cid, ((g, b, n16, ncu), own) in enumerate(zip(uni_calls,
                                                          owners)):
            rows_e, cols_e, tils_e = per_core_seg[(m, g, b)]
            c = len(rows_e)
            for ci in range(ncu):
                lo, hi = ci * P, min((ci + 1) * P, c)
                for t in own[ci]:
                    sc = np.full(P, -1.0, dtype=np.float32)
                    if hi > lo:
                        seg_t = tils_e[lo:hi]
                        msk = seg_t == t
                        if msk.any():
                            sc[np.arange(lo, hi)[msk] - lo] = \
                                cols_e[lo:hi][msk]
                    colx[:, scol_of[(t, jidx[t])]] = sc
                    jidx[t] += 1

        x_own = np.zeros((SHP, IN), dtype=np.float32)
        deg_own = np.ones(SHP, dtype=np.float32)
        mask_own = np.zeros(SHP, dtype=np.float32)
        sl = slice(m * SH, (m + 1) * SH)
        pos_m = node_pos[sl]
        x_own[pos_m] = x[sl]
        deg_own[pos_m] = deg[sl]
        mask_own[pos_m] = 1.0

        iota4 = np.tile(np.arange(P, dtype=np.float32), (P, 4))

        in_maps.append({
            "xtl": np.ascontiguousarray(
                x_own.reshape(T, P, IN).transpose(1, 0, 2).reshape(P, T * IN)),
            "xT": np.ascontiguousarray(x_own.T.astype(_BF16)),
            "deg": np.ascontiguousarray(deg_own.reshape(T, P).T),
            "mask": np.ascontiguousarray(mask_own.reshape(T, P).T),
            "gidx": np.ascontiguousarray(gidx_w),
            "colx": np.ascontiguousarray(colx),
            "iota4": np.ascontiguousarray(iota4),
            "W": np.ascontiguousarray(W.astype(_BF16)),
            "skipW": np.ascontiguousarray(skip_W.astype(_BF16)),
            "gamma": np.ascontiguousarray(
                gamma.astype(np.float32).reshape(1, OUT)),
            "beta": np.ascontiguousarray(
                beta.astype(np.float32).reshape(1, OUT)),
        })
        layouts.append({"calls": list(uni_calls),
                        "call_slices": call_slices,
                        "tile_prog": tile_prog, "NS": NS,
                        "gidx_cols": gidx_w.shape[1],
                        "colx_cols": colx.shape[1]})
    return in_maps, layouts, SH, T, SHP, NFP, NB, BK, NG, node_pos


def _pad_inmaps(in_maps, layouts):
    """Pad per-core gidx/colx arrays to a common width so one set of dram
    tensor shapes serves all cores (SPMD)."""
    gw = max(l["gidx_cols"] for l in layouts)
    cw = max(l["colx_cols"] for l in layouts)
    cw = -(-cw // 4) * 4
    for im, l in zip(in_maps, layouts):
        g = im["gidx"]
        if g.shape[1] < gw:
            im["gidx"] = np.ascontiguousarray(
                np.concatenate([g, np.zeros((P, gw - g.shape[1]), np.int16)],
                               axis=1))
        c = im["colx"]
        if c.shape[1] < cw:
            im["colx"] = np.ascontiguousarray(
                np.concatenate([c, np.full((P, cw - c.shape[1]), -1.0,
                                           np.float32)], axis=1))
    return gw, cw


def _build(M, N, IN, OUT, T, NB, GT, layouts, GW, CW):
    from concourse import bacc, mybir, tile
    from concourse.masks import make_identity

    dt = mybir.dt
    Alu = mybir.AluOpType
    Act = mybir.ActivationFunctionType

    SHP = T * P
    NFP = M * SHP
    BK = NFP // NB
    IN2 = 2 * IN
    BN_EPS = 1e-5
    NG = -(-T // GT)

    nc = bacc.Bacc("TRN2", target_bir_lowering=False, debug=False,
                   num_devices=M)

    xtl_d = nc.dram_tensor("xtl", [P, T * IN], dt.float32,
                           kind="ExternalInput")
    xT_d = nc.dram_tensor("xT", [IN, SHP], dt.bfloat16, kind="ExternalInput")
    deg_d = nc.dram_tensor("deg", [P, T], dt.float32, kind="ExternalInput")
    mask_d = nc.dram_tensor("mask", [P, T], dt.float32, kind="ExternalInput")
    gidx_d = nc.dram_tensor("gidx", [P, GW], dt.int16, kind="ExternalInput")
    colx_d = nc.dram_tensor("colx", [P, CW], dt.float32,
                            kind="ExternalInput")
    iota4_d = nc.dram_tensor("iota4", [P, 4 * P], dt.float32,
                             kind="ExternalInput")
    W_d = nc.dram_tensor("W", [IN, OUT], dt.bfloat16, kind="ExternalInput")
    skipW_d = nc.dram_tensor("skipW", [IN, OUT], dt.bfloat16,
                             kind="ExternalInput")
    gamma_d = nc.dram_tensor("gamma", [1, OUT], dt.float32,
                             kind="ExternalInput")
    beta_d = nc.dram_tensor("beta", [1, OUT], dt.float32,
                            kind="ExternalInput")
    out_d = nc.dram_tensor("out", [SHP, OUT], dt.float32,
                           kind="ExternalOutput")

    y_local = nc.dram_tensor("y_local", [SHP, IN2], dt.bfloat16)
    y_full = nc.dram_tensor("y_full", [NFP, IN2], dt.bfloat16)
    st_local = nc.dram_tensor("st_local", [1, 2 * OUT], dt.float32)
    st_global = nc.dram_tensor("st_global", [1, 2 * OUT], dt.float32,
                               addr_space="Shared")

    rg = [list(range(M))]
    lay = layouts[0]   # SPMD: all cores share structure of core 0? NO —
    # layouts differ per core; SPMD requires ONE program. We use core 0's
    # structure ONLY where identical; per-core variable parts (call sizes,
    # tile programs) must be THE SAME across cores for a single SPMD
    # program. They are not — so we take the MAX/union approach: pad all
    # cores to core-0's... Instead: require same structure via layout
    # normalization done in kernel() (see _normalize_layouts).

    with tile.TileContext(nc) as tc:
        with (
            tc.tile_pool(name="const", bufs=1) as cpool,
            tc.tile_pool(name="xload", bufs=3) as xpool,
            tc.tile_pool(name="gidxp", bufs=2) as gxpool,
            tc.tile_pool(name="gather", bufs=2) as gpool,
            tc.tile_pool(name="sel", bufs=4) as spool,
            tc.tile_pool(name="evac", bufs=3) as epool,
            tc.tile_pool(name="outt", bufs=2) as opool,
            tc.tile_pool(name="ps_agg", bufs=2, space="PSUM") as ps_agg,
            tc.tile_pool(name="ps_tr", bufs=2, space="PSUM") as ps_tr,
            tc.tile_pool(name="ps_out", bufs=2, space="PSUM") as ps_out,
        ):
            W_sb = cpool.tile([IN, OUT], dt.bfloat16, tag="W")
            nc.sync.dma_start(W_sb[:], W_d[:, :])
            skipW_sb = cpool.tile([IN, OUT], dt.bfloat16, tag="skipW")
            nc.sync.dma_start(skipW_sb[:], skipW_d[:, :])
            iota4_sb = cpool.tile([P, 4, P], dt.float32, tag="iota4")
            nc.sync.dma_start(iota4_sb[:], iota4_d[:, :])
            deg_sb = cpool.tile([P, T], dt.float32, tag="deg")
            nc.sync.dma_start(deg_sb[:], deg_d[:, :])
            mask_sb = cpool.tile([P, T], dt.float32, tag="mask")
            nc.sync.dma_start(mask_sb[:], mask_d[:, :])
            colx_sb = cpool.tile([P, CW], dt.float32, tag="colx")
            nc.sync.dma_start(colx_sb[:], colx_d[:, :])
            xT_sb = cpool.tile([IN, SHP], dt.bfloat16, tag="xT")
            nc.sync.dma_start(xT_sb[:], xT_d[:, :])
            gamma_sb = cpool.tile([1, OUT], dt.float32, tag="gamma")
            nc.sync.dma_start(gamma_sb[:], gamma_d[:, :])
            beta_sb = cpool.tile([1, OUT], dt.float32, tag="beta")
            nc.sync.dma_start(beta_sb[:], beta_d[:, :])

            ident = cpool.tile([P, P], dt.float32, tag="ident")
            make_identity(nc, ident[:])
            identb = cpool.tile([P, P], dt.bfloat16, tag="identb")
            nc.vector.tensor_copy(identb[:], ident[:])
            ones_col = cpool.tile([P, 1], dt.float32, tag="ones_col")
            nc.vector.memset(ones_col[:], 1.0)
            ones_row = cpool.tile([1, P], dt.float32, tag="ones_row")
            nc.vector.memset(ones_row[:], 1.0)

            y_sb = cpool.tile([P, T, IN2], dt.bfloat16, tag="y_sb")
            vbuf = cpool.tile([P, T * OUT], dt.float32, tag="vbuf")
            acc_sum = cpool.tile([P, OUT], dt.float32, tag="acc_sum")
            acc_sq = cpool.tile([P, OUT], dt.float32, tag="acc_sq")

            dinv_sb = cpool.tile([P, T], dt.float32, tag="dinv")
            rec_t = cpool.tile([P, T], dt.float32, tag="rec_t")
            nc.vector.reciprocal(rec_t[:], deg_sb[:])
            nc.scalar.activation(dinv_sb[:], rec_t[:], Act.Sqrt)

            # ---- phase A: y = dinv*x -> bf16 hi/lo in y_sb + DRAM ----
            # x loaded in 14-tile slabs; y AllGathered in two halves so
            # gathers on the first half's banks start while AG2 flies
            BT = 14
            TH = T // 2
            HS = TH * P
            HF = M * HS

            def phase_a(t_lo, t_hi):
                for t0 in range(t_lo, t_hi, BT):
                    nb_ = min(BT, t_hi - t0)
                    xt_ = xpool.tile([P, BT, IN], dt.float32, tag="xt_")
                    nc.sync.dma_start(xt_[:, 0:nb_, :],
                                      xtl_d[:, t0 * IN:(t0 + nb_) * IN])
                    y32s = xpool.tile([P, BT, IN], dt.float32, tag="y32s")
                    dslc = dinv_sb[:, t0:t0 + nb_]
                    nc.vector.tensor_tensor(
                        y32s[:, 0:nb_, :], xt_[:, 0:nb_, :],
                        dslc.unsqueeze(2).broadcast_to((P, nb_, IN)),
                        Alu.mult)
                    nc.vector.tensor_copy(y_sb[:, t0:t0 + nb_, 0:IN],
                                          y32s[:, 0:nb_, :])
                    nc.vector.tensor_tensor(
                        y_sb[:, t0:t0 + nb_, IN:IN2], y32s[:, 0:nb_, :],
                        y_sb[:, t0:t0 + nb_, 0:IN], Alu.subtract)
                    for k in range(nb_):
                        t = t0 + k
                        nc.sync.dma_start(y_local[t * P:(t + 1) * P, :],
                                          y_sb[:, t, :])

            phase_a(0, TH)
            nc.gpsimd.collective_compute(
                "AllGather", Alu.bypass, replica_groups=rg,
                ins=[y_local[0:HS, :].opt()],
                outs=[y_full[0:HF, :].opt()])
            phase_a(TH, T)
            nc.gpsimd.collective_compute(
                "AllGather", Alu.bypass, replica_groups=rg,
                ins=[y_local[HS:SHP, :].opt()],
                outs=[y_full[HF:NFP, :].opt()])

            # ---- main loop over groups ----
            calls = lay["calls"]
            call_slices = lay["call_slices"]
            tile_prog = lay["tile_prog"]
            calls_of_group = [[] for _ in range(NG)]
            for cid, (g, b, n16, ncol) in enumerate(calls):
                calls_of_group[g].append(cid)

            GXW = max(w16 for (_, w16) in call_slices)
            GCOL = max(ncol for (_, _, _, ncol) in calls)
            gbuf = {}    # call_id -> (tile, ncols)
            for g in range(NG):
                for cid in calls_of_group[g]:
                    (gg, b, n16, ncol) = calls[cid]
                    off16, w16 = call_slices[cid]
                    gx = gxpool.tile([P, GXW], dt.int16, tag=f"gx{b}")
                    nc.sync.dma_start(gx[:, 0:w16],
                                      gidx_d[:, off16:off16 + w16])
                    Gb = gpool.tile([P, GCOL, IN2], dt.bfloat16, tag=f"G{b}")
                    nc.gpsimd.dma_gather(
                        Gb[:, 0:ncol, :], y_full[b * BK:(b + 1) * BK, :],
                        gx[:, 0:w16], n16, n16, IN2,
                        single_packet=(n16 <= 1024))
                    gbuf[cid] = (Gb, ncol)

                t0 = g * GT
                for t in range(t0, min(t0 + GT, T)):
                    off, ents = tile_prog[t]
                    ne = len(ents)
                    pagg = ps_agg.tile([P, IN2], dt.float32, tag="pagg")
                    # self-loop: pagg = I^T @ y_tile
                    nc.tensor.matmul(pagg[:], lhsT=identb[:],
                                     rhs=y_sb[:, t, :],
                                     start=True, stop=(ne == 0))
                    # batched x4 one-hot builds: one DVE op per 4 chunks
                    # (colx is 4-aligned per tile; pad columns are all -1)
                    Sts = []
                    for si in range(-(-ne // 4)):
                        S4 = spool.tile([P, 4, P], dt.bfloat16, tag="S4")
                        o4 = off + si * 4
                        cslc = colx_sb[:, o4:o4 + 4]
                        nc.vector.tensor_tensor(
                            S4[:], iota4_sb[:],
                            cslc.unsqueeze(2).broadcast_to((P, 4, P)),
                            Alu.is_equal)
                        Sts.append(S4)
                    for j, (cid, ci) in enumerate(ents):
                        Gb, ncol = gbuf[cid]
                        nc.tensor.matmul(pagg[:],
                                         lhsT=Sts[j // 4][:, j % 4, :],
                                         rhs=Gb[:, ci, :],
                                         start=False, stop=(j == ne - 1))

                    aggs = epool.tile([P, IN], dt.float32, tag="aggs")
                    nc.vector.tensor_copy(aggs[:], pagg[:, 0:IN])
                    nc.vector.tensor_tensor(aggs[:], aggs[:],
                                            pagg[:, IN:IN2], Alu.add)
                    agg = epool.tile([P, IN], dt.float32, tag="agg")
                    nc.vector.tensor_scalar(
                        agg[:], aggs[:], dinv_sb[:, t:t + 1], None, Alu.mult)
                    paggT = ps_tr.tile([IN, P], dt.float32, tag="paggT")
                    nc.tensor.transpose(paggT[:], agg[:], ident[:])
                    aggT = epool.tile([IN, P], dt.bfloat16, tag="aggT")
                    nc.vector.tensor_copy(aggT[:], paggT[:])

                    pout = ps_out.tile([P, OUT], dt.float32, tag="pout")
                    nc.tensor.matmul(pout[:], lhsT=aggT[:], rhs=W_sb[:],
                                     start=True, stop=False)
                    nc.tensor.matmul(pout[:],
                                     lhsT=xT_sb[:, t * P:(t + 1) * P],
                                     rhs=skipW_sb[:], start=False, stop=True)
                    v = vbuf[:, t * OUT:(t + 1) * OUT]
                    nc.vector.tensor_scalar(
                        v, pout[:], mask_sb[:, t:t + 1], None, Alu.mult)
                    sq = epool.tile([P, OUT], dt.float32, tag="sq")
                    nc.vector.tensor_tensor(sq[:], v, v, Alu.mult)
                    if t == 0:
                        nc.vector.tensor_copy(acc_sum[:], v)
                        nc.vector.tensor_copy(acc_sq[:], sq[:])
                    else:
                        nc.vector.tensor_tensor(acc_sum[:], acc_sum[:], v,
                                                Alu.add)
                        nc.vector.tensor_tensor(acc_sq[:], acc_sq[:], sq[:],
                                                Alu.add)

            # ---- BN stats + apply + relu ----
            pst1 = ps_agg.tile([1, OUT], dt.float32, tag="pagg")
            nc.tensor.matmul(pst1[:], lhsT=ones_col[:], rhs=acc_sum[:],
                             start=True, stop=True)
            pst2 = ps_tr.tile([1, OUT], dt.float32, tag="paggT")
            nc.tensor.matmul(pst2[:], lhsT=ones_col[:], rhs=acc_sq[:],
                             start=True, stop=True)
            st_sb = cpool.tile([1, 2 * OUT], dt.float32, tag="st_sb")
            nc.scalar.copy(st_sb[:, 0:OUT], pst1[:])
            nc.scalar.copy(st_sb[:, OUT:2 * OUT], pst2[:])
            nc.sync.dma_start(st_local[:, :], st_sb[:])
            nc.gpsimd.collective_compute(
                "AllReduce", Alu.add, replica_groups=rg,
                ins=[st_local.ap().opt()], outs=[st_global.ap().opt()])
            sg_sb = cpool.tile([1, 2 * OUT], dt.float32, tag="sg_sb")
            nc.sync.dma_start(sg_sb[:], st_global[:, :])

            inv_n = 1.0 / float(N)
            mean_sb = cpool.tile([1, OUT], dt.float32, tag="mean_sb")
            nc.vector.tensor_scalar(mean_sb[:], sg_sb[:, 0:OUT], inv_n, None,
                                    Alu.mult)
            var_sb = cpool.tile([1, OUT], dt.float32, tag="var_sb")
            nc.vector.tensor_scalar(var_sb[:], sg_sb[:, OUT:2 * OUT], inv_n,
                                    None, Alu.mult)
            msq = cpool.tile([1, OUT], dt.float32, tag="msq")
            nc.vector.tensor_tensor(msq[:], mean_sb[:], mean_sb[:], Alu.mult)
            nc.vector.tensor_tensor(var_sb[:], var_sb[:], msq[:],
                                    Alu.subtract)
            nc.vector.tensor_scalar(var_sb[:], var_sb[:], BN_EPS, None,
                                    Alu.add)
            rvar = cpool.tile([1, OUT], dt.float32, tag="rvar")
            nc.vector.reciprocal(rvar[:], var_sb[:])
            rstd = cpool.tile([1, OUT], dt.float32, tag="rstd")
            nc.scalar.activation(rstd[:], rvar[:], Act.Sqrt)

            ab_sb = cpool.tile([1, 2 * OUT], dt.float32, tag="ab_sb")
            nc.vector.tensor_tensor(ab_sb[:, 0:OUT], gamma_sb[:], rstd[:],
                                    Alu.mult)
            ma = cpool.tile([1, OUT], dt.float32, tag="ma")
            nc.vector.tensor_tensor(ma[:], mean_sb[:], ab_sb[:, 0:OUT],
                                    Alu.mult)
            nc.vector.tensor_tensor(ab_sb[:, OUT:2 * OUT], beta_sb[:], ma[:],
                                    Alu.subtract)

            prep = ps_out.tile([P, 2 * OUT], dt.float32, tag="prep")
            nc.tensor.matmul(prep[:], lhsT=ones_row[:], rhs=ab_sb[:],
                             start=True, stop=True)
            BO = 4
            a_rep = cpool.tile([P, BO * OUT], dt.float32, tag="a_rep")
            b_rep = cpool.tile([P, BO * OUT], dt.float32, tag="b_rep")
            for k in range(BO):
                nc.scalar.copy(a_rep[:, k * OUT:(k + 1) * OUT],
                               prep[:, 0:OUT])
                nc.scalar.copy(b_rep[:, k * OUT:(k + 1) * OUT],
                               prep[:, OUT:2 * OUT])

            for t0 in range(0, T, BO):
                nb_ = min(BO, T - t0)
                v = vbuf[:, t0 * OUT:(t0 + nb_) * OUT]
                o1 = opool.tile([P, BO * OUT], dt.float32, tag="o1")
                nc.vector.tensor_tensor(
                    o1[:, 0:nb_ * OUT], v, a_rep[:, 0:nb_ * OUT], Alu.mult)
                nc.vector.tensor_tensor(
                    o1[:, 0:nb_ * OUT], o1[:, 0:nb_ * OUT],
                    b_rep[:, 0:nb_ * OUT], Alu.add)
                o2 = opool.tile([P, BO * OUT], dt.float32, tag="o2")
                nc.scalar.activation(o2[:, 0:nb_ * OUT], o1[:, 0:nb_ * OUT],
                                     Act.Relu)
                for k in range(nb_):
                    t = t0 + k
                    nc.sync.dma_start(
                        out_d[t * P:(t + 1) * P, :],
                        o2[:, k * OUT:(k + 1) * OUT])

    nc.compile()
    return nc


def _normalize_layouts(in_maps, layouts):
    """SPMD needs one program for all cores: pad every core's call list and
    tile programs to core-0-compatible structure. We instead rebuild each
    core's data so that the STRUCTURE (call count/sizes per group, per-tile
    program lengths) equals the element-wise maximum across cores, padding
    with inactive entries (idx=-1 rows / colx=-1 columns)."""
    M = len(layouts)
    NGc = max(max((g for (g, b, n, c) in l["calls"]), default=0)
              for l in layouts) + 1
    # unify call structure per (g, b): nidx16/ncols = max over cores
    sizes = {}
    for l in layouts:
        for (g, b, n16, nc_) in l["calls"]:
            k = (g, b)
            n0, c0 = sizes.get(k, (0, 0))
            sizes[k] = (max(n0, n16), max(c0, nc_))
    # per-tile program length = max over cores
    T = len(layouts[0]["tile_prog"])
    plen = [0] * T
    for l in layouts:
        for t in range(T):
            plen[t] = max(plen[t], len(l["tile_prog"][t][1]))
    return sizes, plen, NGc


def kernel(x, edge_index, W, bias, skip_W, gamma, beta, _trace=False,
           _return_results=False):
    x = np.asarray(x, dtype=np.float32)
    edge_index = np.asarray(edge_index, dtype=np.int32)
    M = 8
    N, IN = x.shape
    OUT = np.asarray(W).shape[1]
    GT = 5

    in_maps, layouts, SH, T, SHP, NFP, NB, BK, NG, node_pos = _host_prep(
        x, edge_index, W, skip_W, gamma, beta, M, IN, OUT, GT)

    # structure is union-built in _host_prep: identical across cores
    GW, CW = _pad_inmaps(in_maps, layouts)
    for im in in_maps:
        assert im["gidx"].shape[1] == GW and im["colx"].shape[1] == CW

    structure = tuple(layouts[0]["calls"]) + tuple(
        (o, len(e)) for o, e in layouts[0]["tile_prog"])
    key = (M, N, IN, OUT, T, NB, GT, GW, CW, hash(structure))
    if key not in _KCACHE:
        _KCACHE[key] = _build(M, N, IN, OUT, T, NB, GT, layouts, GW, CW)
    nc = _KCACHE[key]

    from concourse import bass_utils
    res = bass_utils.run_bass_kernel_spmd(
        nc, in_maps, core_ids=list(range(M)), trace=_trace)
    outs = [res.results[m]["out"][node_pos[m * SH:(m + 1) * SH]]
            for m in range(M)]
    full = np.concatenate(outs, axis=0).astype(np.float32)
    if _return_results:
        return full, res
    return full
